# revision 8
# baseline (speedup 1.0000x reference)
"""Trainium2 Bass kernel for nn_BRASKModel (B=4, L=128, R=48, H=768, D=100).

Sharding: core = (direction, batch). Cores 0-3 compute the f-direction for
batches 0-3; cores 4-7 the b-direction. Each core produces a (128, 98) slab
[head_s, head_e, span_s (48), span_e (48)]; the host concatenates the f/b
slabs into the (4, 128, 196) full output (matching the reference's 8-tuple
concatenated along the last axis).

All model FLOPs run on device. The span gather is rewritten as a band-matrix
matmul (BandT[i,j] = 1 iff i-j == s_idx) built from on-device comparisons;
the broadcast attention e = tanh(proj_x + proj_g + proj_r) runs as a DVE
broadcast-add + one big ACT tanh per h-chunk, reduced against V by PE
matmuls accumulating in PSUM.
"""

import numpy as np
import ml_dtypes

BF16 = ml_dtypes.bfloat16
F16 = np.float16
L, H, R, NCH = 128, 768, 48, 6  # NCH = H/128 chunks
RH = 24                          # r per half

_CACHE = {}


def _build_nc(use_f16=True):
    import concourse.bass as bass
    import concourse.tile as tile
    from concourse import mybir, bacc
    from contextlib import ExitStack

    f32 = mybir.dt.float32
    bf = mybir.dt.bfloat16
    f16 = mybir.dt.float16 if use_f16 else mybir.dt.bfloat16
    A = mybir.AluOpType
    ACT = mybir.ActivationFunctionType

    nc = bacc.Bacc(None, target_bir_lowering=False, debug=False)

    def din(name, shape, dt):
        return nc.dram_tensor(name, shape, dt, kind="ExternalInput")

    x_bf = din("x_bf", [L, H], bf)          # X[b]
    xt_bf = din("xt_bf", [H, L], bf)        # X[b]^T
    xt_f32 = din("xt_f32", [H, L], f32)     # X[b]^T fp32 (exact heads)
    wa = din("wa", [H, H], bf)              # W4|W7
    w0 = din("w0", [H, H], bf)              # W0|W1
    wc = din("wc", [H, H], bf)              # W8|W9
    wcb = din("wcb", [1, H], bf)            # b8|b9
    wr = din("wr", [H, H], bf)              # W2|W5
    wrb = din("wrb", [1, H], bf)            # b2|b5
    wg = din("wg", [H, H], bf)              # W3|W6
    wgb = din("wgb", [1, H], bf)            # (b3+b4)|(b6+b7)
    hb = din("hb", [H, 1], bf)              # h_gs[b]
    fcs_w = din("fcs_w", [H, 2], f32)       # start/end head weights (f32)
    fcs_b = din("fcs_b", [1, 2], f32)
    fco_w = din("fco_w", [H, 2], bf)        # output head weights
    fco_b = din("fco_b", [1, 2], bf)
    vw = din("vw", [H, 1], f16)             # fc_w[8]
    b0c = din("b0c", [H, 1], f32)           # b0|b1
    relproj = din("relproj", [128, H], bf)  # [r_proj_w; r_proj_b; 0] | zeros
    relpre = din("relpre", [128, R], bf)    # [transe^T; ones; 0] | zeros
    reldir = din("reldir", [H, R], f32)     # f_rel^T | zeros
    dmat = din("dmat", [128, 128], f32)     # D[i,j] = i - j
    ident = din("ident", [128, 128], f32)
    iota_r = din("iota_r", [1, 128], f32)
    iotam_r = din("iotam_r", [1, 128], f32)  # iota - 128
    ones_rb = din("ones_rb", [1, 128], bf)
    ones_rf = din("ones_rf", [1, 128], f32)
    ones_cb = din("ones_cb", [128, 1], bf)
    ones2f = din("ones2f", [2, 128], f32)

    out = nc.dram_tensor("out", [L, 98], f32, kind="ExternalOutput")

    with tile.TileContext(nc) as tc, ExitStack() as ctx:
        P = ctx.enter_context  # pools
        persist = P(tc.tile_pool(name="persist", bufs=1))
        work = P(tc.tile_pool(name="work", bufs=2))
        minis = P(tc.tile_pool(name="minis", bufs=1))
        ps_big = P(tc.tile_pool(name="ps_big", bufs=1, space="PSUM"))
        ps_gt = P(tc.tile_pool(name="ps_gt", bufs=1, space="PSUM"))
        ps_med = P(tc.tile_pool(name="ps_med", bufs=2, space="PSUM"))
        ps_sm = P(tc.tile_pool(name="ps_sm", bufs=2, space="PSUM"))
        ps_v = P(tc.tile_pool(name="ps_v", bufs=1, space="PSUM"))

        def load(name, dram_ap, shape, dt, pool=persist):
            t = pool.tile(shape, dt, tag=name, name=name)
            nc.sync.dma_start(out=t[:], in_=dram_ap)
            return t

        # ---- SBUF loads (DRAM -> SBUF) ----
        ch = lambda ap: ap.rearrange("(c p) h -> p c h", p=128)  # noqa: E731
        sb_wg = load("sb_wg", ch(wg[:]), [128, NCH, H], bf)
        sb_wgb = load("sb_wgb", wgb[:], [1, H], bf)
        sb_xtb = load("sb_xtb", ch(xt_bf[:]), [128, NCH, L], bf)
        sb_xtf = load("sb_xtf", ch(xt_f32[:]), [128, NCH, L], f32)
        sb_wa = load("sb_wa", ch(wa[:]), [128, NCH, H], bf)
        sb_wr = load("sb_wr", ch(wr[:]), [128, NCH, H], bf)
        sb_wrb = load("sb_wrb", wrb[:], [1, H], bf)
        sb_w0 = load("sb_w0", ch(w0[:]), [128, NCH, H], bf)
        sb_wc = load("sb_wc", ch(wc[:]), [128, NCH, H], bf)
        sb_wcb = load("sb_wcb", wcb[:], [1, H], bf)
        sb_x = load("sb_x", x_bf[:], [L, H], bf)
        sb_hb = load("sb_hb", ch(hb[:]), [128, NCH, 1], bf)
        sb_fcs = load("sb_fcs", ch(fcs_w[:]), [128, NCH, 2], f32)
        sb_fcsb = load("sb_fcsb", fcs_b[:], [1, 2], f32)
        sb_fco = load("sb_fco", ch(fco_w[:]), [128, NCH, 2], bf)
        sb_fcob = load("sb_fcob", fco_b[:], [1, 2], bf)
        sb_vw = load("sb_vw", ch(vw[:]), [128, NCH, 1], f16)
        sb_b0 = load("sb_b0", ch(b0c[:]), [128, NCH, 1], f32)
        sb_rp = load("sb_rp", relproj[:], [128, H], bf)
        sb_rpre = load("sb_rpre", relpre[:], [128, R], bf)
        sb_rdir = load("sb_rdir", ch(reldir[:]), [128, NCH, R], f32)
        sb_d = load("sb_d", dmat[:], [128, 128], f32)
        sb_id = load("sb_id", ident[:], [128, 128], f32)
        sb_iota = load("sb_iota", iota_r[:], [1, 128], f32)
        sb_iotam = load("sb_iotam", iotam_r[:], [1, 128], f32)
        sb_1rb = load("sb_1rb", ones_rb[:], [1, 128], bf)
        sb_1rf = load("sb_1rf", ones_rf[:], [1, 128], f32)
        sb_1cb = load("sb_1cb", ones_cb[:], [128, 1], bf)

        NS = [(0, 512), (512, 256)]  # N-splits for H-wide psum outputs

        # ---- 1. q = h_gs[b] @ Wg + (bg + ba)  -> (1, H) ----
        q_ps = ps_big.tile([1, H], f32, tag="big", name="q")
        for n0, nn in NS:
            for k in range(7):
                lhsT = sb_hb[:, k, :] if k < 6 else sb_1rb[:, 0:1]
                rhs = sb_wg[:, k, n0:n0 + nn] if k < 6 else sb_wgb[:, n0:n0 + nn]
                nc.tensor.matmul(q_ps[:, n0:n0 + nn], lhsT, rhs,
                                 start=(k == 0), stop=(k == 6))
        q_sb = persist.tile([1, H], bf, tag="q_sb", name="q_sb")
        nc.scalar.copy(q_sb[:], q_ps[:])

        # ---- 2. xg = X @ Wa + q  -> (L, H), then transpose to (H, L) ----
        xg_ps = ps_big.tile([128, H], f32, tag="big", name="big")
        for n0, nn in NS:
            for k in range(7):
                lhsT = sb_xtb[:, k, :] if k < 6 else sb_1rb[:]
                rhs = sb_wa[:, k, n0:n0 + nn] if k < 6 else q_sb[:, n0:n0 + nn]
                nc.tensor.matmul(xg_ps[:, n0:n0 + nn], lhsT, rhs,
                                 start=(k == 0), stop=(k == 6))
        xg16 = persist.tile([128, H], f16, tag="xg16", name="xg16")
        nc.vector.tensor_copy(xg16[:], xg_ps[:])
        sb_xgT = persist.tile([128, NCH, 128], f16, tag="sb_xgT", name="sb_xgT")
        for c in range(NCH):
            nc.sync.dma_start_transpose(sb_xgT[:, c, :], xg16[:, c * 128:(c + 1) * 128])

        # ---- 3. heads hsT = (X @ fcs + fcsb)^T -> (2, L) f32 ----
        hs_sbs = []
        for j in range(2):
            hs_ps = ps_sm.tile([1, 128], f32, tag="sm", name=f"hs{j}")
            for k in range(7):
                lhsT = sb_fcs[:, k, j:j + 1] if k < 6 else sb_fcsb[:, j:j + 1]
                rhs = sb_xtf[:, k, :] if k < 6 else sb_1rf[:]
                nc.tensor.matmul(hs_ps[:], lhsT, rhs, start=(k == 0), stop=(k == 6))
            hs_sb_j = persist.tile([1, 128], f32, tag=f"hs_sb{j}", name=f"hs_sb{j}")
            nc.vector.tensor_copy(hs_sb_j[:], hs_ps[:])
            hs_sbs.append(hs_sb_j)
        hs0_sb, hs1_sb = hs_sbs

        # ---- 4. span extraction (tiny DVE ops on partition 0) ----
        def mini(tag, shape=(1, 1)):
            return minis.tile(list(shape), f32, tag=tag, name=tag)

        smask = mini("smask", (1, 128))
        nc.vector.tensor_scalar(smask[:], hs0_sb[:], 0.0, None, op0=A.is_gt)
        sc1 = mini("sc1", (1, 128))
        nc.vector.tensor_mul(sc1[:], smask[:], sb_iotam[:])
        scand = mini("scand", (1, 128))
        nc.vector.tensor_scalar_add(scand[:], sc1[:], 128.0)
        sidx = mini("sidx")
        nc.vector.tensor_reduce(sidx[:], scand[:], axis=mybir.AxisListType.X, op=A.min)
        emask = mini("emask", (1, 128))
        nc.vector.tensor_scalar(emask[:], hs1_sb[:], 0.0, None, op0=A.is_gt)
        posge = mini("posge", (1, 128))
        nc.vector.tensor_scalar(posge[:], sb_iota[:], sidx[:], None, op0=A.is_ge)
        ec1 = mini("ec1", (1, 128))
        nc.vector.tensor_mul(ec1[:], emask[:], posge[:])
        ec2 = mini("ec2", (1, 128))
        nc.vector.tensor_mul(ec2[:], ec1[:], sb_iotam[:])
        ecand = mini("ecand", (1, 128))
        nc.vector.tensor_scalar_add(ecand[:], ec2[:], 128.0)
        emin = mini("emin")
        nc.vector.tensor_reduce(emin[:], ecand[:], axis=mybir.AxisListType.X, op=A.min)
        hase = mini("hase")
        nc.vector.tensor_scalar(hase[:], emin[:], 128.0, None, op0=A.is_lt)
        hass = mini("hass")
        nc.vector.tensor_scalar(hass[:], sidx[:], 128.0, None, op0=A.is_lt)
        ed = mini("ed")
        nc.vector.tensor_sub(ed[:], emin[:], sidx[:])
        ed2 = mini("ed2")
        nc.vector.tensor_mul(ed2[:], ed[:], hase[:])
        spl = mini("spl")
        nc.vector.tensor_scalar_add(spl[:], ed2[:], 1.0)
        mask_row = mini("mask_row", (1, 128))
        nc.vector.tensor_scalar(mask_row[:], sb_iota[:], spl[:], hass[:],
                                op0=A.is_lt, op1=A.mult)
        # broadcasts to 128 partitions via ones-column matmuls
        mb_ps = ps_sm.tile([128, 128], f32, tag="sm", name="mb")
        nc.tensor.matmul(mb_ps[:], sb_1rf[:], mask_row[:], start=True, stop=True)
        m_bc = persist.tile([128, 128], f32, tag="m_bc", name="m_bc")
        nc.vector.tensor_copy(m_bc[:], mb_ps[:])
        sx_ps = ps_sm.tile([128, 1], f32, tag="sm", name="sx")
        nc.tensor.matmul(sx_ps[:], sb_1rf[:], sidx[:], start=True, stop=True)
        sidx_col = persist.tile([128, 1], f32, tag="sidx_col", name="sidx_col")
        nc.vector.tensor_copy(sidx_col[:], sx_ps[:])
        bandT = persist.tile([128, 128], bf, tag="bandT", name="bandT")
        nc.vector.tensor_scalar(bandT[:], sb_d[:], sidx_col[:], None, op0=A.is_equal)

        # ---- 5. xw0 = X @ W0 -> (L, H) ----
        xw0_ps = ps_big.tile([128, H], f32, tag="big", name="big")
        for n0, nn in NS:
            for k in range(6):
                nc.tensor.matmul(xw0_ps[:, n0:n0 + nn], sb_xtb[:, k, :],
                                 sb_w0[:, k, n0:n0 + nn], start=(k == 0), stop=(k == 5))
        xw0_sb = persist.tile([128, H], bf, tag="xw0_sb", name="xw0_sb")
        nc.vector.tensor_copy(xw0_sb[:], xw0_ps[:])

        # ---- 6. xw8 = X @ Wc + bc -> (L, H), transpose -> (H, L) ----
        xw8_ps = ps_big.tile([128, H], f32, tag="big", name="big")
        for n0, nn in NS:
            for k in range(7):
                lhsT = sb_xtb[:, k, :] if k < 6 else sb_1rb[:]
                rhs = sb_wc[:, k, n0:n0 + nn] if k < 6 else sb_wcb[:, n0:n0 + nn]
                nc.tensor.matmul(xw8_ps[:, n0:n0 + nn], lhsT, rhs,
                                 start=(k == 0), stop=(k == 6))
        xw8_sb = persist.tile([128, H], bf, tag="xw8_sb", name="xw8_sb")
        nc.vector.tensor_copy(xw8_sb[:], xw8_ps[:])
        sb_xw8T = persist.tile([128, NCH, 128], bf, tag="sb_xw8T", name="sb_xw8T")
        for c in range(NCH):
            nc.sync.dma_start_transpose(sb_xw8T[:, c, :], xw8_sb[:, c * 128:(c + 1) * 128])

        # ---- 7. relT = (proj^T @ pre) + reldir -> (H, R) chunks ----
        relT_ps = ps_med.tile([128, NCH, R], f32, tag="med48", name="med48")
        for c in range(NCH):
            nc.tensor.matmul(relT_ps[:, c, :], sb_rp[:, c * 128:(c + 1) * 128],
                             sb_rpre[:], start=True, stop=True)
        relT_bf = persist.tile([128, NCH, R], bf, tag="relT_bf", name="relT_bf")
        nc.vector.tensor_add(relT_bf[:], relT_ps[:], sb_rdir[:])

        # ---- 8. prT[h, r] = (rel @ Wr + br)^T directly -> (H, R) chunks ----
        prT_ps = ps_med.tile([128, NCH, R], f32, tag="med48", name="med48")
        for c in range(NCH):
            for k in range(7):
                lhsT = sb_wr[:, k, c * 128:(c + 1) * 128] if k < 6 else sb_wrb[:, c * 128:(c + 1) * 128]
                rhs = relT_bf[:, k, :] if k < 6 else sb_1rb[:, 0:R]
                nc.tensor.matmul(prT_ps[:, c, :], lhsT, rhs,
                                 start=(k == 0), stop=(k == 6))
        sb_prT = persist.tile([128, NCH, R], f16, tag="sb_prT", name="sb_prT")
        nc.vector.tensor_copy(sb_prT[:], prT_ps[:])

        # ---- 9. G^T chunks + HikXT = m*(G^T+b0) + xw8^T + X^T (bf16) ----
        sb_hikxt = persist.tile([128, NCH, 128], bf, tag="sb_hikxt", name="sb_hikxt")
        for c in range(NCH):
            gt_c = ps_gt.tile([128, 128], f32, tag="gtc", name="gtc")
            nc.tensor.matmul(gt_c[:], xw0_sb[:, c * 128:(c + 1) * 128],
                             bandT[:], start=True, stop=True)
            hx1 = work.tile([128, 128], f32, tag="hx1", name="hx1")
            nc.vector.scalar_tensor_tensor(hx1[:], gt_c[:], sb_b0[:, c, :],
                                           m_bc[:], op0=A.add, op1=A.mult)
            hx2 = work.tile([128, 128], f32, tag="hx2", name="hx2")
            nc.vector.tensor_add(hx2[:], hx1[:], sb_xw8T[:, c, :])
            nc.vector.tensor_add(sb_hikxt[:, c, :], hx2[:], sb_xtb[:, c, :])

        # ---- 10. t1 = fco^T @ HikXT + fcob -> (2, L) ----
        t1_sbs = []
        for j in range(2):
            t1_ps = ps_sm.tile([1, 128], f32, tag="sm", name=f"t1_{j}")
            for k in range(7):
                lhsT = sb_fco[:, k, j:j + 1] if k < 6 else sb_fcob[:, j:j + 1]
                rhs = sb_hikxt[:, k, :] if k < 6 else sb_1rb[:]
                nc.tensor.matmul(t1_ps[:], lhsT, rhs, start=(k == 0), stop=(k == 6))
            t1_sb_j = persist.tile([1, 128], f32, tag=f"t1_sb{j}", name=f"t1_sb{j}")
            nc.vector.tensor_copy(t1_sb_j[:], t1_ps[:])
            t1_sbs.append(t1_sb_j)
        t10_sb, t11_sb = t1_sbs

        # ---- 11. e-loop: t = xgT + prT (bcast), e = tanh(t), v += e^T vw ----
        v_ps = ps_v.tile([128, R], f32, tag="v", name="v")
        for rh in range(2):
            e_all = work.tile([128, NCH, RH, 128], f16, tag="e_all", name="e_all")
            for c in range(NCH):
                t_t = work.tile([128, RH, 128], f16, tag="t_t", name="t_t")
                in0 = sb_xgT[:, c, :].unsqueeze(1).broadcast_to((128, RH, 128))
                in1 = sb_prT[:, c, rh * RH:(rh + 1) * RH].unsqueeze(2).broadcast_to((128, RH, 128))
                nc.vector.tensor_tensor(t_t[:], in0, in1, op=A.add)
                nc.scalar.activation(e_all[:, c], t_t[:], ACT.Tanh)
            for r in range(RH):
                for c in range(NCH):
                    nc.tensor.matmul(v_ps[:, rh * RH + r:rh * RH + r + 1],
                                     e_all[:, c, r, :], sb_vw[:, c, :],
                                     start=(c == 0), stop=(c == 5))

        # ---- 12. softmax pieces: E = exp(v); den; unnormalized C^T ----
        E_sb = persist.tile([128, R], f32, tag="E_sb", name="E_sb")
        nc.scalar.activation(E_sb[:], v_ps[:], ACT.Exp)
        E_bf = persist.tile([128, R], bf, tag="E_bf", name="E_bf")
        nc.vector.tensor_copy(E_bf[:], E_sb[:])
        den_ps = ps_sm.tile([R, 1], f32, tag="sm", name="den")
        nc.tensor.matmul(den_ps[:], E_bf[:], sb_1cb[:], start=True, stop=True)
        rden = persist.tile([R, 1], f32, tag="rden", name="rden")
        nc.vector.reciprocal(rden[:], den_ps[:])
        rdT_ps = ps_sm.tile([1, R], f32, tag="sm", name="rdT")
        nc.tensor.transpose(rdT_ps[:], rden[:], sb_id[0:R, 0:R])
        rdT_sb = persist.tile([1, R], f32, tag="rdT_sb", name="rdT_sb")
        nc.vector.tensor_copy(rdT_sb[:], rdT_ps[:])

        ct_ps = ps_med.tile([128, NCH, R], f32, tag="med48", name="med48")
        for c in range(NCH):
            nc.tensor.matmul(ct_ps[:, c, :], sb_x[:, c * 128:(c + 1) * 128],
                             E_bf[:], start=True, stop=True)
        ct_bf = persist.tile([128, NCH, R], bf, tag="ct_bf", name="ct_bf")
        nc.vector.tensor_copy(ct_bf[:], ct_ps[:])

        # ---- 13. t2 = (fco^T @ C^T_u) * rden -> (2, R) ----
        t2_sbs = []
        for j in range(2):
            t2_ps = ps_sm.tile([1, R], f32, tag="sm", name=f"t2_{j}")
            for k in range(6):
                nc.tensor.matmul(t2_ps[:], sb_fco[:, k, j:j + 1], ct_bf[:, k, :],
                                 start=(k == 0), stop=(k == 5))
            t2_sb_j = persist.tile([1, R], f32, tag=f"t2_sb{j}", name=f"t2_sb{j}")
            nc.vector.tensor_mul(t2_sb_j[:], t2_ps[:], rdT_sb[:])
            t2_sbs.append(t2_sb_j)
        t20_sb, t21_sb = t2_sbs

        # ---- 14. final assembly ----
        asm_lS = load("asm_lS", ones2f[:], [2, 128], f32)
        nc.vector.tensor_copy(asm_lS[0:1, :], t10_sb[:])
        asm_lE = load("asm_lE", ones2f[:], [2, 128], f32)
        nc.vector.tensor_copy(asm_lE[0:1, :], t11_sb[:])
        asm_rS = load("asm_rS", ones2f[:, 0:R], [2, R], f32)
        nc.sync.dma_start(out=asm_rS[1:2, :], in_=t20_sb[:])
        asm_rE = load("asm_rE", ones2f[:, 0:R], [2, R], f32)
        nc.sync.dma_start(out=asm_rE[1:2, :], in_=t21_sb[:])

        outS_ps = ps_sm.tile([128, R], f32, tag="sm", name="outS")
        nc.tensor.matmul(outS_ps[:], asm_lS[:], asm_rS[:], start=True, stop=True)
        outE_ps = ps_sm.tile([128, R], f32, tag="sm", name="outE")
        nc.tensor.matmul(outE_ps[:], asm_lE[:], asm_rE[:], start=True, stop=True)
        hh0_ps = ps_sm.tile([128, 1], f32, tag="sm", name="hh0")
        nc.tensor.transpose(hh0_ps[:], hs0_sb[:], sb_id[0:1, 0:1])
        hh1_ps = ps_sm.tile([128, 1], f32, tag="sm", name="hh1")
        nc.tensor.transpose(hh1_ps[:], hs1_sb[:], sb_id[0:1, 0:1])

        out_sb = persist.tile([128, 98], f32, tag="out_sb", name="out_sb")
        nc.vector.tensor_copy(out_sb[:, 0:1], hh0_ps[:])
        nc.vector.tensor_copy(out_sb[:, 1:2], hh1_ps[:])
        nc.vector.tensor_copy(out_sb[:, 2:50], outS_ps[:])
        nc.vector.tensor_copy(out_sb[:, 50:98], outE_ps[:])
        nc.sync.dma_start(out=out[:], in_=out_sb[:])

    nc.compile()
    return nc


def get_nc():
    if "nc" not in _CACHE:
        _CACHE["nc"] = _build_nc()
    return _CACHE["nc"]


def prep_in_maps(inputs):
    """Build the 8 per-core input maps from the full-model inputs."""
    tok = np.asarray(inputs["token_embs"], np.float32)
    h_gs = np.asarray(inputs["h_gs"], np.float32)
    f_rel = np.asarray(inputs["f_rel_embs"], np.float32)
    transe = np.asarray(inputs["b_rel_transe"], np.float32)
    rpw = np.asarray(inputs["r_proj_w"], np.float32)
    rpb = np.asarray(inputs["r_proj_b"], np.float32)
    fc_w = np.asarray(inputs["fc_w"], np.float32)
    fc_b = np.asarray(inputs["fc_b"], np.float32)
    big_w = np.asarray(inputs["big_w"], np.float32)
    big_b = np.asarray(inputs["big_b"], np.float32)

    iota = np.arange(128, dtype=np.float32)
    consts = {
        "dmat": (iota[:, None] - iota[None, :]).astype(np.float32),
        "ident": np.eye(128, dtype=np.float32),
        "iota_r": iota[None, :].copy(),
        "iotam_r": (iota - 128.0)[None, :].copy(),
        "ones_rb": np.ones((1, 128), BF16),
        "ones_rf": np.ones((1, 128), np.float32),
        "ones_cb": np.ones((128, 1), BF16),
        "ones2f": np.ones((2, 128), np.float32),
        "vw": fc_w[8][:, None].astype(F16),
    }
    rp_b = np.zeros((128, H), np.float32)
    rp_b[:100] = rpw
    rp_b[100] = rpb
    rpre_b = np.zeros((128, R), np.float32)
    rpre_b[:100] = transe.T
    rpre_b[100] = 1.0
    zeros128H = np.zeros((128, H), BF16)
    zeros128R = np.zeros((128, R), BF16)
    zerosHR = np.zeros((H, R), np.float32)

    in_maps = []
    for c in range(8):
        d, b = c // 4, c % 4
        X = tok[b]
        if d == 0:
            ia, ig, i0, ic, ir = 4, 3, 0, 2, 8  # wait indices below
        # index map: f: Wa=4 Wg=3 W0=0 Wc=8 Wr=2 ; b: Wa=7 Wg=6 W0=1 Wc=9 Wr=5
        if d == 0:
            ia, ig, i0, ic, ir = 4, 3, 0, 8, 2
            fcs_i, fco_i = (0, 1), (2, 3)
            relproj_c, relpre_c = zeros128H, zeros128R
            reldir_c = np.ascontiguousarray(f_rel.T)
        else:
            ia, ig, i0, ic, ir = 7, 6, 1, 9, 5
            fcs_i, fco_i = (4, 5), (6, 7)
            relproj_c = rp_b.astype(BF16)
            relpre_c = rpre_b.astype(BF16)
            reldir_c = zerosHR
        m = {
            "x_bf": X.astype(BF16),
            "xt_bf": np.ascontiguousarray(X.T).astype(BF16),
            "xt_f32": np.ascontiguousarray(X.T),
            "wa": big_w[ia].astype(BF16),
            "w0": big_w[i0].astype(BF16),
            "wc": big_w[ic].astype(BF16),
            "wcb": big_b[ic][None, :].astype(BF16),
            "wr": big_w[ir].astype(BF16),
            "wrb": big_b[ir][None, :].astype(BF16),
            "wg": big_w[ig].astype(BF16),
            "wgb": (big_b[ig] + big_b[ia])[None, :].astype(BF16),
            "hb": h_gs[b][:, None].astype(BF16),
            "fcs_w": np.ascontiguousarray(fc_w[list(fcs_i)].T),
            "fcs_b": fc_b[list(fcs_i)][None, :].astype(np.float32),
            "fco_w": np.ascontiguousarray(fc_w[list(fco_i)].T).astype(BF16),
            "fco_b": fc_b[list(fco_i)][None, :].astype(BF16),
            "b0c": big_b[i0][:, None].astype(np.float32),
            "relproj": relproj_c,
            "relpre": relpre_c,
            "reldir": reldir_c,
        }
        m.update(consts)
        in_maps.append(m)
    return in_maps


def assemble(results):
    out = np.empty((4, L, 196), np.float32)
    for b in range(4):
        out[b, :, 0:98] = results[b]["out"]
        out[b, :, 98:196] = results[4 + b]["out"]
    return out


def kernel(**inputs):
    from concourse.bass_utils import run_bass_kernel_spmd
    nc = get_nc()
    in_maps = prep_in_maps(inputs)
    res = run_bass_kernel_spmd(nc, in_maps, core_ids=list(range(8)))
    return assemble(res.results)


if __name__ == "__main__":
    nc = get_nc()
    print("build + compile OK")
